# revision 15
# baseline (speedup 1.0000x reference)
"""GAT (2-layer, 8-head) Trainium2 kernel over 8 NeuronCores.

Strategy (edge-cut node sharding, v5):
- Degree-balanced node relabeling (LPT into 392 blocks of 128). Core c
  owns blocks [49c, 49(c+1)); each block's in-edges are split into
  lo-source (node id < 25088) and hi-source chunks of 128 so gather
  indices fit int16, then padded to global (NLOC, NHIC) chunk counts.
- Edge gathers use the batched SWDGE dma_gather (InstDMAGatherAnt):
  512 indices per instruction, packed descriptors -> ~2x the service
  rate of per-chunk indirect DMAs. Layer-1 table rows are 640 f16
  (1280B, 256B-aligned): [Wh 512 | f_src_hi 8 | f_src_lo 8 | pad].
  Layer-2 rows are 128 f16 (256B): [Wh2 64 | f2_hi | f2_lo | pad].
- Per-edge f_dst comes from a per-chunk PE matmul D = OHT.T @ fd where
  OHT (transposed one-hot) is host-precomputed; OH is built on device
  via is_equal(iota, rl). p=exp(leaky(fsrc+D)); R=p*Wh;
  pnum += OH.T@R, pden += OH.T@p accumulate in PSUM per dst block.
- Phase A computes the layer-1 table with fp32r matmuls (full PE rate);
  AllGather between phases; phase C mirrors B at width 64.
- Softmax needs no segment-max: logits are O(6) so fp16 exp is safe,
  and normalization commutes with the scatter-sum.
"""
import os
import sys
sys.path.insert(0, "/opt/trn_rl_repo")
import numpy as np

import concourse.tile as tile
from concourse import bass, bacc, mybir
from concourse.bass_utils import run_bass_kernel_spmd
from concourse.masks import make_identity

N, E = 50000, 800000
NFEAT, NHID, NHEADS, NCLASS = 512, 64, 8, 64
ALPHA = 0.2
NC = 8
NPAD = 50176
SHARD = NPAD // NC        # 6272
LOSH = 3200               # lo rows per shard (25 blocks)
HISH = SHARD - LOSH       # 3072
LOSZ = NC * LOSH          # 25600 global lo rows
HISZ = NC * HISH          # 24576
BLK = 128
NBPC = SHARD // BLK       # 49 blocks per core
NBLK = NPAD // BLK        # 392 blocks total
KT = NFEAT // 128         # 4 k-tiles
DW1 = 640                 # f16 row: Wh 512 | fsrc_hi 8 | fsrc_lo 8 | pad 112
DW2 = 128                 # f16 row: Wh2 64 | f2_hi | f2_lo | pad 62
XW = KT * 128             # 512
GCH = 4                   # chunks per dma_gather call (512 idxs)

f16d, f32d, f32r, i16d, i32d = (mybir.dt.float16, mybir.dt.float32,
                                mybir.dt.float32r, mybir.dt.int16,
                                mybir.dt.int32)

LAST_EXEC_NS = None
LAST_RESULTS = None
_BUILD_CACHE = {}


def _preprocess(row, col):
    """Degree-balanced relabeling + lo/hi chunk building."""
    import heapq
    deg = np.bincount(row, minlength=NPAD)
    order = np.argsort(-deg, kind="stable")
    heap = [(0, b) for b in range(NBLK)]
    heapq.heapify(heap)
    fill = np.zeros(NBLK, np.int32)
    perm = np.empty(NPAD, np.int64)
    loads = np.zeros(NBLK, np.int64)
    for v in order:
        while True:
            load, b = heapq.heappop(heap)
            if fill[b] < BLK:
                break
        perm[v] = b * BLK + fill[b]
        fill[b] += 1
        loads[b] = load + deg[v]
        if fill[b] < BLK:
            heapq.heappush(heap, (loads[b], b))

    row_n = perm[row]
    col_n = perm[col]
    o = np.argsort(row_n, kind="stable")
    row_s, col_s = row_n[o], col_n[o]
    counts = np.bincount(row_s // BLK, minlength=NBLK)
    starts = np.concatenate([[0], np.cumsum(counts)])

    # per-block lo/hi edge lists
    lo_ids, lo_rls, hi_ids, hi_rls = [], [], [], []
    nlo_max = nhi_max = 0
    for b in range(NBLK):
        s, e = starts[b], starts[b + 1]
        cs = col_s[s:e]
        cc_ = cs // SHARD
        rr_ = cs % SHARD
        tr = np.where(rr_ < LOSH, cc_ * LOSH + rr_,
                      LOSZ + cc_ * HISH + (rr_ - LOSH))
        ls = (row_s[s:e] - b * BLK)
        m = tr < LOSZ
        lo_ids.append(tr[m]); lo_rls.append(ls[m])
        hi_ids.append(tr[~m] - LOSZ); hi_rls.append(ls[~m])
        nlo_max = max(nlo_max, int(m.sum()))
        nhi_max = max(nhi_max, int((~m).sum()))
    NLOC = -(-nlo_max // 128)
    NHIC = -(-nhi_max // 128)
    cpb = NLOC + NHIC
    ncht = NBPC * cpb

    ids_arr = np.zeros((NC, ncht, 128), np.int16)
    rl = np.full((NC, 128, ncht), 200.0, np.float16)
    for b in range(NBLK):
        c, bl = divmod(b, NBPC)
        for ids, rls, chunk0, nch in ((lo_ids[b], lo_rls[b], bl * cpb, NLOC),
                                      (hi_ids[b], hi_rls[b],
                                       bl * cpb + NLOC, NHIC)):
            n = len(ids)
            pad = nch * 128 - n
            idp = np.concatenate([ids, np.zeros(pad, ids.dtype)])
            rlp = np.concatenate([rls, np.full(pad, 200, rls.dtype)])
            ids_arr[c, chunk0:chunk0 + nch] = idp.reshape(nch, 128)
            rl[c, :, chunk0:chunk0 + nch] = rlp.reshape(nch, 128).T
    # wrap: idx16[r, ch*8+g] = ids_arr[ch, g*16+r]
    idx16 = (ids_arr.reshape(NC, ncht, 8, 16)
             .transpose(0, 3, 1, 2)           # [NC, 16, ncht, 8]
             .reshape(NC, 16, ncht * 8))
    idx16 = np.tile(idx16, (1, 8, 1)).astype(np.int16)   # [NC, 128, ncht*8]

    oht = np.zeros((NC, 128, ncht * 128), np.float16)
    rli = rl.astype(np.int32)
    cc, ee, ch = np.nonzero(rli < 128)
    dv = rli[cc, ee, ch]
    oht[cc, dv, ch * 128 + ee] = 1.0
    return perm, rl, idx16, oht, NLOC, NHIC


def _build(NLOC, NHIC):
    key = (NLOC, NHIC)
    if key in _BUILD_CACHE:
        return _BUILD_CACHE[key]
    cpb = NLOC + NHIC
    ncht = NBPC * cpb
    nc = bacc.Bacc("TRN2", target_bir_lowering=False, debug=False,
                   enable_asserts=True, num_devices=NC, num_swdge_queues=4)
    xt = nc.dram_tensor("xt", [NBPC, 128, XW], f32d, kind="ExternalInput")
    w1 = nc.dram_tensor("w1", [KT * 128, 528], f32d, kind="ExternalInput")
    w2 = nc.dram_tensor("w2", [KT * 128, 66], f32d, kind="ExternalInput")
    rl = nc.dram_tensor("rl", [128, ncht], f16d, kind="ExternalInput")
    idx = nc.dram_tensor("idx", [128, ncht * 8], i16d, kind="ExternalInput")
    ohtd = nc.dram_tensor("ohtd", [128, ncht * 128], f16d, kind="ExternalInput")
    iotar_in = nc.dram_tensor("iotar", [128, 128], f16d, kind="ExternalInput")
    out = nc.dram_tensor("out", [SHARD, NCLASS], f32d, kind="ExternalOutput")

    AF, ALU = mybir.ActivationFunctionType, mybir.AluOpType

    def gather_calls(gp, Gt, tabs, dw, ch0):
        """Emit lo+hi dma_gather calls for one block into tile Gt."""
        q = 0
        for tab, nch, goff in ((tabs[0], NLOC, 0), (tabs[1], NHIC, NLOC)):
            for g0 in range(0, nch, GCH):
                k = min(GCH, nch - g0)
                gc = goff + g0                    # chunk within block
                cg = ch0 + gc                     # global chunk
                nc.gpsimd.dma_gather(
                    Gt[:, gc * dw:(gc + k) * dw]
                        .rearrange("p (c d) -> p c d", c=k),
                    tab[:, :],
                    idx_t[:, cg * 8:(cg + k) * 8],
                    k * 128, k * 128, dw, queue_num=q)
                q = (q + 1) % 4

    with tile.TileContext(nc) as tc:
        with tc.tile_pool(name="res", bufs=1) as res, \
             tc.tile_pool(name="dram", bufs=1, space="DRAM") as drp:
            tab1s = drp.tile([SHARD, DW1], f16d)
            tab1a = drp.tile([LOSZ, DW1], f16d, addr_space="Shared")
            tab1b = drp.tile([HISZ, DW1], f16d, addr_space="Shared")
            fda = drp.tile([SHARD, 16], f16d)
            fdc = drp.tile([SHARD, 2], f16d)
            tab2s = drp.tile([SHARD, DW2], f16d)
            tab2a = drp.tile([LOSZ, DW2], f16d, addr_space="Shared")
            tab2b = drp.tile([HISZ, DW2], f16d, addr_space="Shared")

            w1_t = res.tile([128, KT * 528], f32r)
            w2_t = res.tile([128, KT * 66], f32r)
            for k in range(KT):
                nc.sync.dma_start(out=w1_t[:, k * 528:(k + 1) * 528],
                                  in_=w1[k * 128:(k + 1) * 128, :].bitcast(f32r))
                nc.sync.dma_start(out=w2_t[:, k * 66:(k + 1) * 66],
                                  in_=w2[k * 128:(k + 1) * 128, :].bitcast(f32r))
            rl_t = res.tile([128, ncht], f16d)
            nc.sync.dma_start(out=rl_t[:], in_=rl[:, :])
            idx_t = res.tile([128, ncht * 8], i16d)
            nc.sync.dma_start(out=idx_t[:], in_=idx[:, :])
            iot = res.tile([128, 128], f16d)
            nc.sync.dma_start(out=iot[:], in_=iotar_in[:, :])
            ident = res.tile([128, 128], f32d)
            make_identity(nc, ident[:])

            # ---------------- Phase A ----------------
            with nc.named_scope("phaseA"), \
                 tc.tile_pool(name="pa", bufs=3) as pa, \
                 tc.tile_pool(name="ppa", bufs=2, space="PSUM") as ppa:
                for nt in range(NBPC):
                    rows = slice(nt * 128, (nt + 1) * 128)
                    psA = ppa.tile([128, 512], f32d, tag="psA")
                    psB = ppa.tile([128, 16], f32d, tag="psB")
                    XK = pa.tile([128, XW], f32r, tag="XK")
                    nc.sync.dma_start(out=XK[:], in_=xt[nt, :, :].bitcast(f32r))
                    for k in range(KT):
                        nc.tensor.matmul(
                            out=psA[:], lhsT=XK[:, k * 128:(k + 1) * 128],
                            rhs=w1_t[:, k * 528:k * 528 + 512],
                            start=(k == 0), stop=(k == KT - 1))
                        nc.tensor.matmul(
                            out=psB[:], lhsT=XK[:, k * 128:(k + 1) * 128],
                            rhs=w1_t[:, k * 528 + 512:(k + 1) * 528],
                            start=(k == 0), stop=(k == KT - 1))
                    whf = pa.tile([128, DW1], f16d, tag="whf")
                    nc.scalar.activation(out=whf[:, :512], in_=psA[:],
                                         func=AF.Copy)
                    nc.scalar.activation(out=whf[:, 512:520], in_=psB[:, 8:16],
                                         func=AF.Copy)
                    nc.vector.tensor_tensor(out=whf[:, 520:528], in0=psB[:, 8:16],
                                            in1=whf[:, 512:520], op=ALU.subtract)
                    fdt = pa.tile([128, 16], f16d, tag="fdt")
                    nc.scalar.activation(out=fdt[:, 0:8], in_=psB[:, 0:8],
                                         func=AF.Copy)
                    nc.vector.tensor_tensor(out=fdt[:, 8:16], in0=psB[:, 0:8],
                                            in1=fdt[:, 0:8], op=ALU.subtract)
                    nc.sync.dma_start(out=tab1s[rows, :], in_=whf[:])
                    nc.sync.dma_start(out=fda[rows, :], in_=fdt[:])

            with nc.named_scope("ag1"):
                nc.gpsimd.collective_compute(
                    "AllGather", ALU.bypass, replica_groups=[list(range(NC))],
                    ins=[tab1s[0:LOSH, :].opt()], outs=[tab1a.opt()])
                nc.gpsimd.collective_compute(
                    "AllGather", ALU.bypass, replica_groups=[list(range(NC))],
                    ins=[tab1s[LOSH:SHARD, :].opt()],
                    outs=[tab1b.opt()])

            # ---------------- Phase B ----------------
            with nc.named_scope("phaseB"), \
                 tc.tile_pool(name="pg", bufs=3) as pg, \
                 tc.tile_pool(name="pot", bufs=2) as pot, \
                 tc.tile_pool(name="pwk", bufs=2) as pwk, \
                 tc.tile_pool(name="pep", bufs=2) as pep, \
                 tc.tile_pool(name="ppb", bufs=1, space="PSUM") as ppb, \
                 tc.tile_pool(name="ppd", bufs=2, space="PSUM") as ppd, \
                 tc.tile_pool(name="ppt", bufs=2, space="PSUM") as ppt, \
                 tc.tile_pool(name="pps", bufs=1, space="PSUM") as pps:
                for bl in range(NBPC):
                    rows = slice(bl * 128, (bl + 1) * 128)
                    ch0 = bl * cpb
                    pnum = ppb.tile([128, 512], f32d, tag="pnum")
                    pden = ppb.tile([128, 8], f32d, tag="pden")
                    G = pg.tile([128, cpb * DW1], f16d, tag="G")
                    gather_calls(pg, G, (tab1a, tab1b), DW1, ch0)
                    OT = pot.tile([128, cpb * 128], f16d, tag="OT")
                    nc.sync.dma_start(
                        out=OT[:], in_=ohtd[:, ch0 * 128:(ch0 + cpb) * 128])
                    fdb = pot.tile([128, 16], f16d, tag="fdb")
                    nc.sync.dma_start(out=fdb[:], in_=fda[rows, :])
                    Dps = ppd.tile([128, cpb * 8], f32d, tag="Dps")
                    for i in range(cpb):
                        nc.tensor.matmul(out=Dps[:, i * 8:(i + 1) * 8],
                                         lhsT=OT[:, i * 128:(i + 1) * 128],
                                         rhs=fdb[:, 0:8], start=True, stop=False)
                        nc.tensor.matmul(out=Dps[:, i * 8:(i + 1) * 8],
                                         lhsT=OT[:, i * 128:(i + 1) * 128],
                                         rhs=fdb[:, 8:16], start=False, stop=True)
                    OH = pwk.tile([128, cpb * 128], f16d, tag="OH")
                    nc.vector.tensor_tensor(
                        out=OH[:].rearrange("p (c f) -> p c f", c=cpb),
                        in0=iot[:].rearrange("p (o f) -> p o f", o=1)
                            .to_broadcast([128, cpb, 128]),
                        in1=rl_t[:, ch0:ch0 + cpb].to_broadcast([128, cpb, 128]),
                        op=ALU.is_equal)
                    s1 = pwk.tile([128, cpb * 8], f32d, tag="s1")
                    e1 = pwk.tile([128, cpb * 8], f32d, tag="e1")
                    p16 = pwk.tile([128, cpb * 8], f16d, tag="p16")
                    Gq = G[:].rearrange("p (c d) -> p c d", d=DW1)
                    Dr = Dps[:].rearrange("p (c d) -> p c d", d=8)
                    s1r = s1[:].rearrange("p (c f) -> p c f", c=cpb)
                    e1r = e1[:].rearrange("p (c f) -> p c f", c=cpb)
                    nc.vector.tensor_tensor(out=s1r, in0=Gq[:, :, 512:520],
                                            in1=Gq[:, :, 520:528], op=ALU.add)
                    nc.vector.tensor_tensor(out=e1r, in0=s1r, in1=Dr[:, :, 0:8],
                                            op=ALU.add)
                    nc.vector.tensor_scalar_mul(s1[:], e1[:], ALPHA)
                    nc.vector.tensor_tensor(out=e1[:], in0=e1[:], in1=s1[:],
                                            op=ALU.max)
                    nc.scalar.activation(out=p16[:], in_=e1[:], func=AF.Exp)
                    Px = pwk.tile([128, cpb * 512], f16d, tag="Px")
                    nc.scalar.activation(
                        out=Px[:].rearrange("p (c e f) -> p c e f", c=cpb, e=8),
                        in_=e1[:].rearrange("p (c h) -> p c h", c=cpb)
                            .to_broadcast([128, cpb, 8, 64]),
                        func=AF.Exp)
                    R = pwk.tile([128, cpb * 512], f16d, tag="R")
                    nc.vector.tensor_tensor(
                        out=R[:].rearrange("p (c e f) -> p c e f", c=cpb, e=8),
                        in0=Gq[:, :, 0:512].rearrange("p c (e f) -> p c e f", e=8),
                        in1=Px[:].rearrange("p (c e f) -> p c e f", c=cpb, e=8),
                        op=ALU.mult)
                    for i in range(cpb):
                        nc.tensor.matmul(out=pnum[:],
                                         lhsT=OH[:, i * 128:(i + 1) * 128],
                                         rhs=R[:, i * 512:(i + 1) * 512],
                                         start=(i == 0), stop=(i == cpb - 1))
                        nc.tensor.matmul(out=pden[:],
                                         lhsT=OH[:, i * 128:(i + 1) * 128],
                                         rhs=p16[:, i * 8:(i + 1) * 8],
                                         start=(i == 0), stop=(i == cpb - 1))
                    # epilogue: h = elu(num/den), transpose, layer-2 tables
                    dcl = pep.tile([128, 8], f32d, tag="dcl")
                    nc.vector.tensor_scalar_max(dcl[:], pden[:], 1e-30)
                    nc.vector.reciprocal(out=dcl[:], in_=dcl[:])
                    h = pep.tile([128, 512], f32d, tag="h")
                    nc.vector.tensor_tensor(
                        out=h[:].rearrange("p (e f) -> p e f", e=8),
                        in0=pnum[:].rearrange("p (e f) -> p e f", e=8),
                        in1=dcl[:].to_broadcast([128, 8, 64]),
                        op=ALU.mult)
                    hm = pep.tile([128, 512], f32d, tag="hm")
                    nc.vector.tensor_scalar_min(hm[:], h[:], 0.0)
                    nc.scalar.activation(out=hm[:], in_=hm[:], func=AF.Exp)
                    nc.vector.tensor_scalar_sub(hm[:], hm[:], 1.0)
                    nc.vector.tensor_tensor(out=h[:], in0=hm[:], in1=h[:],
                                            op=ALU.max)
                    ps2 = pps.tile([128, 66], f32d, tag="ps2")
                    for k in range(KT):
                        pt = ppt.tile([128, 128], f32d, tag="pt")
                        nc.tensor.transpose(out=pt[:],
                                            in_=h[:, k * 128:(k + 1) * 128],
                                            identity=ident[:])
                        ht = pep.tile([128, 128], f32r, tag="ht")
                        nc.scalar.activation(out=ht[:], in_=pt[:], func=AF.Copy)
                        nc.tensor.matmul(
                            out=ps2[:], lhsT=ht[:],
                            rhs=w2_t[:, k * 66:(k + 1) * 66],
                            start=(k == 0), stop=(k == KT - 1))
                    t2 = pep.tile([128, DW2], f16d, tag="t2")
                    nc.scalar.activation(out=t2[:, 0:64], in_=ps2[:, 0:64],
                                         func=AF.Copy)
                    nc.scalar.activation(out=t2[:, 64:65], in_=ps2[:, 65:66],
                                         func=AF.Copy)
                    nc.vector.tensor_tensor(out=t2[:, 65:66], in0=ps2[:, 65:66],
                                            in1=t2[:, 64:65], op=ALU.subtract)
                    fd2 = pep.tile([128, 2], f16d, tag="fd2")
                    nc.scalar.activation(out=fd2[:, 0:1], in_=ps2[:, 64:65],
                                         func=AF.Copy)
                    nc.vector.tensor_tensor(out=fd2[:, 1:2], in0=ps2[:, 64:65],
                                            in1=fd2[:, 0:1], op=ALU.subtract)
                    nc.sync.dma_start(out=tab2s[rows, :], in_=t2[:])
                    nc.sync.dma_start(out=fdc[rows, :], in_=fd2[:])

            with nc.named_scope("ag2"):
                nc.gpsimd.collective_compute(
                    "AllGather", ALU.bypass, replica_groups=[list(range(NC))],
                    ins=[tab2s[0:LOSH, :].opt()], outs=[tab2a.opt()])
                nc.gpsimd.collective_compute(
                    "AllGather", ALU.bypass, replica_groups=[list(range(NC))],
                    ins=[tab2s[LOSH:SHARD, :].opt()],
                    outs=[tab2b.opt()])

            # ---------------- Phase C ----------------
            with nc.named_scope("phaseC"), \
                 tc.tile_pool(name="pg2", bufs=3) as pg2, \
                 tc.tile_pool(name="pot2", bufs=2) as pot2, \
                 tc.tile_pool(name="pwk2", bufs=2) as pwk2, \
                 tc.tile_pool(name="ppc", bufs=1, space="PSUM") as ppc, \
                 tc.tile_pool(name="ppd2", bufs=2, space="PSUM") as ppd2:
                for bl in range(NBPC):
                    rows = slice(bl * 128, (bl + 1) * 128)
                    ch0 = bl * cpb
                    ps3 = ppc.tile([128, 65], f32d, tag="ps3")
                    G2 = pg2.tile([128, cpb * DW2], f16d, tag="G2")
                    gather_calls(pg2, G2, (tab2a, tab2b), DW2, ch0)
                    OT2 = pot2.tile([128, cpb * 128], f16d, tag="OT2")
                    nc.sync.dma_start(
                        out=OT2[:], in_=ohtd[:, ch0 * 128:(ch0 + cpb) * 128])
                    fd2b = pot2.tile([128, 2], f16d, tag="fd2b")
                    nc.sync.dma_start(out=fd2b[:], in_=fdc[rows, :])
                    D2ps = ppd2.tile([128, cpb], f32d, tag="D2ps")
                    for i in range(cpb):
                        nc.tensor.matmul(out=D2ps[:, i:i + 1],
                                         lhsT=OT2[:, i * 128:(i + 1) * 128],
                                         rhs=fd2b[:, 0:1], start=True, stop=False)
                        nc.tensor.matmul(out=D2ps[:, i:i + 1],
                                         lhsT=OT2[:, i * 128:(i + 1) * 128],
                                         rhs=fd2b[:, 1:2], start=False, stop=True)
                    OH2 = pwk2.tile([128, cpb * 128], f16d, tag="OH2")
                    nc.vector.tensor_tensor(
                        out=OH2[:].rearrange("p (c f) -> p c f", c=cpb),
                        in0=iot[:].rearrange("p (o f) -> p o f", o=1)
                            .to_broadcast([128, cpb, 128]),
                        in1=rl_t[:, ch0:ch0 + cpb].to_broadcast([128, cpb, 128]),
                        op=ALU.is_equal)
                    e2 = pwk2.tile([128, cpb], f32d, tag="e2")
                    t2c = pwk2.tile([128, cpb], f32d, tag="t2c")
                    p2 = pwk2.tile([128, cpb], f16d, tag="p2")
                    G2r = G2[:].rearrange("p (c d) -> p c d", d=DW2)
                    D2r = D2ps[:].rearrange("p (c d) -> p c d", d=1)
                    e2r = e2[:].rearrange("p (c o) -> p c o", o=1)
                    nc.vector.tensor_tensor(out=e2r, in0=G2r[:, :, 64:65],
                                            in1=G2r[:, :, 65:66], op=ALU.add)
                    nc.vector.tensor_tensor(out=e2r, in0=e2r, in1=D2r[:, :, 0:1],
                                            op=ALU.add)
                    nc.vector.tensor_scalar_mul(t2c[:], e2[:], ALPHA)
                    nc.vector.tensor_tensor(out=e2[:], in0=e2[:], in1=t2c[:],
                                            op=ALU.max)
                    nc.scalar.activation(out=p2[:], in_=e2[:], func=AF.Exp)
                    R2 = pwk2.tile([128, cpb * 65], f16d, tag="R2")
                    R2r = R2[:].rearrange("p (c d) -> p c d", d=65)
                    nc.vector.tensor_tensor(
                        out=R2r[:, :, 0:64],
                        in0=G2r[:, :, 0:64],
                        in1=p2[:].rearrange("p (c o) -> p c o", o=1)
                            .to_broadcast([128, cpb, 64]),
                        op=ALU.mult)
                    nc.scalar.activation(
                        out=R2r[:, :, 64:65],
                        in_=p2[:].rearrange("p (c o) -> p c o", o=1),
                        func=AF.Copy)
                    for i in range(cpb):
                        nc.tensor.matmul(out=ps3[:],
                                         lhsT=OH2[:, i * 128:(i + 1) * 128],
                                         rhs=R2[:, i * 65:(i + 1) * 65],
                                         start=(i == 0), stop=(i == cpb - 1))
                    d2c = pwk2.tile([128, 1], f32d, tag="d2c")
                    nc.vector.tensor_scalar_max(d2c[:], ps3[:, 64:65], 1e-30)
                    nc.vector.reciprocal(out=d2c[:], in_=d2c[:])
                    o = pwk2.tile([128, 64], f32d, tag="o")
                    nc.vector.tensor_tensor(
                        out=o[:].rearrange("p (c f) -> p c f", c=1),
                        in0=ps3[:, 0:64].rearrange("p (c f) -> p c f", c=1),
                        in1=d2c[:].to_broadcast([128, 1, 64]),
                        op=ALU.mult)
                    nc.sync.dma_start(out=out[rows, :], in_=o[:])

    nc.compile()
    _BUILD_CACHE[key] = nc
    return nc


def kernel(**inputs):
    global LAST_EXEC_NS, LAST_RESULTS
    x = inputs["x"].astype(np.float32)
    row = inputs["row"].astype(np.int64)
    col = inputs["col"].astype(np.int64)
    W, a = inputs["W"].astype(np.float32), inputs["a"].astype(np.float32)
    W_out = inputs["W_out"].astype(np.float32)
    a_out = inputs["a_out"].astype(np.float32)

    perm, rl, idx16, oht, NLOC, NHIC = _preprocess(row, col)

    W_cat = np.concatenate([W[h] for h in range(NHEADS)], axis=1)
    WA_dst = np.stack([W[h] @ a[h, :NHID] for h in range(NHEADS)], 1)
    WA_src = np.stack([W[h] @ a[h, NHID:] for h in range(NHEADS)], 1)
    w1_np = np.concatenate([W_cat, WA_dst, WA_src], 1).astype(np.float32)
    w2_np = np.concatenate([W_out, (W_out @ a_out[:NCLASS])[:, None],
                            (W_out @ a_out[NCLASS:])[:, None]], 1).astype(np.float32)

    x_pad = np.zeros((NPAD, NFEAT), np.float32)
    x_pad[perm[:N]] = x
    iotar = np.broadcast_to(np.arange(128, dtype=np.float16), (128, 128)).copy()

    nc = _build(NLOC, NHIC)

    in_maps = []
    for c in range(NC):
        xs = x_pad[c * SHARD:(c + 1) * SHARD]            # [6272, 512]
        xtc = (xs.reshape(NBPC, 128, KT, 128)            # [nt, j, k, p]
                 .transpose(0, 3, 2, 1)                  # [nt, p, k, j]
                 .reshape(NBPC, 128, XW)).copy()
        in_maps.append({"xt": xtc, "w1": w1_np, "w2": w2_np,
                        "rl": rl[c], "idx": idx16[c], "ohtd": oht[c],
                        "iotar": iotar})

    trace = bool(int(os.environ.get("GAT_TRACE", "0")))
    res = run_bass_kernel_spmd(nc, in_maps, list(range(NC)), trace=trace,
                               trace_cores=list(range(NC)) if trace else None)
    LAST_EXEC_NS = res.exec_time_ns
    LAST_RESULTS = res
    out_new = np.concatenate([res.results[c]["out"] for c in range(NC)], 0)
    return out_new[perm[:N]].astype(np.float32)


# revision 16
# speedup vs baseline: 1.0467x; 1.0467x over previous
"""GAT (2-layer, 8-head) Trainium2 kernel over 8 NeuronCores.

Strategy (edge-cut node sharding, v5):
- Degree-balanced node relabeling (LPT into 392 blocks of 128). Core c
  owns blocks [49c, 49(c+1)); each block's in-edges are split into
  lo-source (node id < 25088) and hi-source chunks of 128 so gather
  indices fit int16, then padded to global (NLOC, NHIC) chunk counts.
- Edge gathers use the batched SWDGE dma_gather (InstDMAGatherAnt):
  512 indices per instruction, packed descriptors -> ~2x the service
  rate of per-chunk indirect DMAs. Layer-1 table rows are 640 f16
  (1280B, 256B-aligned): [Wh 512 | f_src_hi 8 | f_src_lo 8 | pad].
  Layer-2 rows are 128 f16 (256B): [Wh2 64 | f2_hi | f2_lo | pad].
- Per-edge f_dst comes from a per-chunk PE matmul D = OHT.T @ fd where
  OHT (transposed one-hot) is host-precomputed; OH is built on device
  via is_equal(iota, rl). p=exp(leaky(fsrc+D)); R=p*Wh;
  pnum += OH.T@R, pden += OH.T@p accumulate in PSUM per dst block.
- Phase A computes the layer-1 table with fp32r matmuls (full PE rate);
  AllGather between phases; phase C mirrors B at width 64.
- Softmax needs no segment-max: logits are O(6) so fp16 exp is safe,
  and normalization commutes with the scatter-sum.
"""
import os
import sys
sys.path.insert(0, "/opt/trn_rl_repo")
import numpy as np

import concourse.tile as tile
from concourse import bass, bacc, mybir
from concourse.bass_utils import run_bass_kernel_spmd
from concourse.masks import make_identity

N, E = 50000, 800000
NFEAT, NHID, NHEADS, NCLASS = 512, 64, 8, 64
ALPHA = 0.2
NC = 8
NPAD = 50176
SHARD = NPAD // NC        # 6272
LOSH = 3200               # lo rows per shard (25 blocks)
HISH = SHARD - LOSH       # 3072
LOSZ = NC * LOSH          # 25600 global lo rows
HISZ = NC * HISH          # 24576
BLK = 128
NBPC = SHARD // BLK       # 49 blocks per core
NBLK = NPAD // BLK        # 392 blocks total
KT = NFEAT // 128         # 4 k-tiles
DW1 = 640                 # f16 row: Wh 512 | fsrc_hi 8 | fsrc_lo 8 | pad 112
DW2 = 128                 # f16 row: Wh2 64 | f2_hi | f2_lo | pad 62
XW = KT * 128             # 512
GCH = 4                   # chunks per dma_gather call (512 idxs)

f16d, f32d, f32r, i16d, i32d = (mybir.dt.float16, mybir.dt.float32,
                                mybir.dt.float32r, mybir.dt.int16,
                                mybir.dt.int32)

LAST_EXEC_NS = None
LAST_RESULTS = None
_BUILD_CACHE = {}


def _preprocess(row, col):
    """Degree-balanced relabeling + lo/hi chunk building."""
    import heapq
    deg = np.bincount(row, minlength=NPAD)
    order = np.argsort(-deg, kind="stable")
    heap = [(0, b) for b in range(NBLK)]
    heapq.heapify(heap)
    fill = np.zeros(NBLK, np.int32)
    perm = np.empty(NPAD, np.int64)
    loads = np.zeros(NBLK, np.int64)
    for v in order:
        while True:
            load, b = heapq.heappop(heap)
            if fill[b] < BLK:
                break
        perm[v] = b * BLK + fill[b]
        fill[b] += 1
        loads[b] = load + deg[v]
        if fill[b] < BLK:
            heapq.heappush(heap, (loads[b], b))

    row_n = perm[row]
    col_n = perm[col]
    o = np.argsort(row_n, kind="stable")
    row_s, col_s = row_n[o], col_n[o]
    counts = np.bincount(row_s // BLK, minlength=NBLK)
    starts = np.concatenate([[0], np.cumsum(counts)])

    # per-block lo/hi edge lists
    lo_ids, lo_rls, hi_ids, hi_rls = [], [], [], []
    nlo_max = nhi_max = 0
    for b in range(NBLK):
        s, e = starts[b], starts[b + 1]
        cs = col_s[s:e]
        cc_ = cs // SHARD
        rr_ = cs % SHARD
        tr = np.where(rr_ < LOSH, cc_ * LOSH + rr_,
                      LOSZ + cc_ * HISH + (rr_ - LOSH))
        ls = (row_s[s:e] - b * BLK)
        m = tr < LOSZ
        lo_ids.append(tr[m]); lo_rls.append(ls[m])
        hi_ids.append(tr[~m] - LOSZ); hi_rls.append(ls[~m])
        nlo_max = max(nlo_max, int(m.sum()))
        nhi_max = max(nhi_max, int((~m).sum()))
    NLOC = -(-nlo_max // 128)
    NHIC = -(-nhi_max // 128)
    cpb = NLOC + NHIC
    ncht = NBPC * cpb

    ids_arr = np.zeros((NC, ncht, 128), np.int16)
    rl = np.full((NC, 128, ncht), 200.0, np.float16)
    for b in range(NBLK):
        c, bl = divmod(b, NBPC)
        for ids, rls, chunk0, nch in ((lo_ids[b], lo_rls[b], bl * cpb, NLOC),
                                      (hi_ids[b], hi_rls[b],
                                       bl * cpb + NLOC, NHIC)):
            n = len(ids)
            pad = nch * 128 - n
            idp = np.concatenate([ids, np.zeros(pad, ids.dtype)])
            rlp = np.concatenate([rls, np.full(pad, 200, rls.dtype)])
            ids_arr[c, chunk0:chunk0 + nch] = idp.reshape(nch, 128)
            rl[c, :, chunk0:chunk0 + nch] = rlp.reshape(nch, 128).T
    # wrap: idx16[r, ch*8+g] = ids_arr[ch, g*16+r]
    idx16 = (ids_arr.reshape(NC, ncht, 8, 16)
             .transpose(0, 3, 1, 2)           # [NC, 16, ncht, 8]
             .reshape(NC, 16, ncht * 8))
    idx16 = np.tile(idx16, (1, 8, 1)).astype(np.int16)   # [NC, 128, ncht*8]

    oht = np.zeros((NC, 128, ncht * 128), np.float16)
    rli = rl.astype(np.int32)
    cc, ee, ch = np.nonzero(rli < 128)
    dv = rli[cc, ee, ch]
    oht[cc, dv, ch * 128 + ee] = 1.0
    return perm, rl, idx16, oht, NLOC, NHIC


def _build(NLOC, NHIC):
    key = (NLOC, NHIC)
    if key in _BUILD_CACHE:
        return _BUILD_CACHE[key]
    cpb = NLOC + NHIC
    ncht = NBPC * cpb
    nc = bacc.Bacc("TRN2", target_bir_lowering=False, debug=False,
                   enable_asserts=True, num_devices=NC, num_swdge_queues=4)
    xt = nc.dram_tensor("xt", [NBPC, 128, XW], f32d, kind="ExternalInput")
    w1 = nc.dram_tensor("w1", [KT * 128, 528], f32d, kind="ExternalInput")
    w2 = nc.dram_tensor("w2", [KT * 128, 66], f32d, kind="ExternalInput")
    rl = nc.dram_tensor("rl", [128, ncht], f16d, kind="ExternalInput")
    idx = nc.dram_tensor("idx", [128, ncht * 8], i16d, kind="ExternalInput")
    ohtd = nc.dram_tensor("ohtd", [128, ncht * 128], f16d, kind="ExternalInput")
    iotar_in = nc.dram_tensor("iotar", [128, 128], f16d, kind="ExternalInput")
    out = nc.dram_tensor("out", [SHARD, NCLASS], f32d, kind="ExternalOutput")

    AF, ALU = mybir.ActivationFunctionType, mybir.AluOpType

    def gather_calls(gp, Gt, tabs, dw, ch0):
        """Emit lo+hi dma_gather calls for one block into tile Gt."""
        q = 0
        for tab, nch, goff in ((tabs[0], NLOC, 0), (tabs[1], NHIC, NLOC)):
            for g0 in range(0, nch, GCH):
                k = min(GCH, nch - g0)
                gc = goff + g0                    # chunk within block
                cg = ch0 + gc                     # global chunk
                nc.gpsimd.dma_gather(
                    Gt[:, gc * dw:(gc + k) * dw]
                        .rearrange("p (c d) -> p c d", c=k),
                    tab[:, :],
                    idx_t[:, cg * 8:(cg + k) * 8],
                    k * 128, k * 128, dw, queue_num=q)
                q = (q + 1) % 4

    with tile.TileContext(nc) as tc:
        with tc.tile_pool(name="res", bufs=1) as res, \
             tc.tile_pool(name="dram", bufs=1, space="DRAM") as drp:
            tab1s = drp.tile([SHARD, DW1], f16d)
            tab1a = drp.tile([LOSZ, DW1], f16d, addr_space="Shared")
            tab1b = drp.tile([HISZ, DW1], f16d, addr_space="Shared")
            fda = drp.tile([SHARD, 16], f16d)
            fdc = drp.tile([SHARD, 2], f16d)
            tab2s = drp.tile([SHARD, DW2], f16d)
            tab2a = drp.tile([LOSZ, DW2], f16d, addr_space="Shared")
            tab2b = drp.tile([HISZ, DW2], f16d, addr_space="Shared")

            w1_t = res.tile([128, KT * 528], f32r)
            w2_t = res.tile([128, KT * 66], f32r)
            for k in range(KT):
                nc.sync.dma_start(out=w1_t[:, k * 528:(k + 1) * 528],
                                  in_=w1[k * 128:(k + 1) * 128, :].bitcast(f32r))
                nc.sync.dma_start(out=w2_t[:, k * 66:(k + 1) * 66],
                                  in_=w2[k * 128:(k + 1) * 128, :].bitcast(f32r))
            rl_t = res.tile([128, ncht], f16d)
            nc.sync.dma_start(out=rl_t[:], in_=rl[:, :])
            idx_t = res.tile([128, ncht * 8], i16d)
            nc.sync.dma_start(out=idx_t[:], in_=idx[:, :])
            iot = res.tile([128, 128], f16d)
            nc.sync.dma_start(out=iot[:], in_=iotar_in[:, :])
            ident = res.tile([128, 128], f32d)
            make_identity(nc, ident[:])

            # ---------------- Phase A ----------------
            with nc.named_scope("phaseA"), \
                 tc.tile_pool(name="pa", bufs=3) as pa, \
                 tc.tile_pool(name="ppa", bufs=2, space="PSUM") as ppa:
                for nt in range(NBPC):
                    rows = slice(nt * 128, (nt + 1) * 128)
                    psA = ppa.tile([128, 512], f32d, tag="psA")
                    psB = ppa.tile([128, 16], f32d, tag="psB")
                    XK = pa.tile([128, XW], f32r, tag="XK")
                    nc.sync.dma_start(out=XK[:], in_=xt[nt, :, :].bitcast(f32r))
                    for k in range(KT):
                        nc.tensor.matmul(
                            out=psA[:], lhsT=XK[:, k * 128:(k + 1) * 128],
                            rhs=w1_t[:, k * 528:k * 528 + 512],
                            start=(k == 0), stop=(k == KT - 1))
                        nc.tensor.matmul(
                            out=psB[:], lhsT=XK[:, k * 128:(k + 1) * 128],
                            rhs=w1_t[:, k * 528 + 512:(k + 1) * 528],
                            start=(k == 0), stop=(k == KT - 1))
                    whf = pa.tile([128, DW1], f16d, tag="whf")
                    nc.scalar.activation(out=whf[:, :512], in_=psA[:],
                                         func=AF.Copy)
                    nc.scalar.activation(out=whf[:, 512:520], in_=psB[:, 8:16],
                                         func=AF.Copy)
                    nc.vector.tensor_tensor(out=whf[:, 520:528], in0=psB[:, 8:16],
                                            in1=whf[:, 512:520], op=ALU.subtract)
                    fdt = pa.tile([128, 16], f16d, tag="fdt")
                    nc.scalar.activation(out=fdt[:, 0:8], in_=psB[:, 0:8],
                                         func=AF.Copy)
                    nc.vector.tensor_tensor(out=fdt[:, 8:16], in0=psB[:, 0:8],
                                            in1=fdt[:, 0:8], op=ALU.subtract)
                    nc.sync.dma_start(out=tab1s[rows, :], in_=whf[:])
                    nc.sync.dma_start(out=fda[rows, :], in_=fdt[:])

            with nc.named_scope("ag1"):
                nc.gpsimd.collective_compute(
                    "AllGather", ALU.bypass, replica_groups=[list(range(NC))],
                    ins=[tab1s[0:LOSH, :].opt()], outs=[tab1a.opt()])
                nc.gpsimd.collective_compute(
                    "AllGather", ALU.bypass, replica_groups=[list(range(NC))],
                    ins=[tab1s[LOSH:SHARD, :].opt()],
                    outs=[tab1b.opt()])

            # ---------------- Phase B ----------------
            with nc.named_scope("phaseB"), \
                 tc.tile_pool(name="pg", bufs=3) as pg, \
                 tc.tile_pool(name="pot", bufs=2) as pot, \
                 tc.tile_pool(name="pwk", bufs=2) as pwk, \
                 tc.tile_pool(name="pep", bufs=2) as pep, \
                 tc.tile_pool(name="ppb", bufs=1, space="PSUM") as ppb, \
                 tc.tile_pool(name="ppd", bufs=2, space="PSUM") as ppd, \
                 tc.tile_pool(name="ppt", bufs=2, space="PSUM") as ppt, \
                 tc.tile_pool(name="pps", bufs=1, space="PSUM") as pps:
                for bl in range(NBPC):
                    rows = slice(bl * 128, (bl + 1) * 128)
                    ch0 = bl * cpb
                    pnum = ppb.tile([128, 512], f32d, tag="pnum")
                    pden = ppb.tile([128, 8], f32d, tag="pden")
                    G = pg.tile([128, cpb * DW1], f16d, tag="G")
                    gather_calls(pg, G, (tab1a, tab1b), DW1, ch0)
                    OT = pot.tile([128, cpb * 128], f16d, tag="OT")
                    nc.sync.dma_start(
                        out=OT[:], in_=ohtd[:, ch0 * 128:(ch0 + cpb) * 128])
                    fdb = pot.tile([128, 16], f16d, tag="fdb")
                    nc.sync.dma_start(out=fdb[:], in_=fda[rows, :])
                    Dps = ppd.tile([128, cpb * 8], f32d, tag="Dps")
                    for i in range(cpb):
                        nc.tensor.matmul(out=Dps[:, i * 8:(i + 1) * 8],
                                         lhsT=OT[:, i * 128:(i + 1) * 128],
                                         rhs=fdb[:, 0:8], start=True, stop=False)
                        nc.tensor.matmul(out=Dps[:, i * 8:(i + 1) * 8],
                                         lhsT=OT[:, i * 128:(i + 1) * 128],
                                         rhs=fdb[:, 8:16], start=False, stop=True)
                    OH = pwk.tile([128, cpb * 128], f16d, tag="OH")
                    nc.vector.tensor_tensor(
                        out=OH[:].rearrange("p (c f) -> p c f", c=cpb),
                        in0=iot[:].rearrange("p (o f) -> p o f", o=1)
                            .to_broadcast([128, cpb, 128]),
                        in1=rl_t[:, ch0:ch0 + cpb].to_broadcast([128, cpb, 128]),
                        op=ALU.is_equal)
                    s1 = pwk.tile([128, cpb * 8], f32d, tag="s1")
                    e1 = pwk.tile([128, cpb * 8], f32d, tag="e1")
                    p16 = pwk.tile([128, cpb * 8], f16d, tag="p16")
                    Gq = G[:].rearrange("p (c d) -> p c d", d=DW1)
                    Dr = Dps[:].rearrange("p (c d) -> p c d", d=8)
                    s1r = s1[:].rearrange("p (c f) -> p c f", c=cpb)
                    e1r = e1[:].rearrange("p (c f) -> p c f", c=cpb)
                    nc.vector.tensor_tensor(out=s1r, in0=Gq[:, :, 512:520],
                                            in1=Gq[:, :, 520:528], op=ALU.add)
                    nc.vector.tensor_tensor(out=e1r, in0=s1r, in1=Dr[:, :, 0:8],
                                            op=ALU.add)
                    nc.vector.tensor_scalar_mul(s1[:], e1[:], ALPHA)
                    nc.vector.tensor_tensor(out=e1[:], in0=e1[:], in1=s1[:],
                                            op=ALU.max)
                    nc.scalar.activation(out=p16[:], in_=e1[:], func=AF.Exp)
                    R = pwk.tile([128, cpb * 512], f16d, tag="R")
                    nc.vector.tensor_tensor(
                        out=R[:].rearrange("p (c e f) -> p c e f", c=cpb, e=8),
                        in0=Gq[:, :, 0:512].rearrange("p c (e f) -> p c e f", e=8),
                        in1=p16[:].rearrange("p (c h) -> p c h", c=cpb)
                            .to_broadcast([128, cpb, 8, 64]),
                        op=ALU.mult)
                    for i in range(cpb):
                        nc.tensor.matmul(out=pnum[:],
                                         lhsT=OH[:, i * 128:(i + 1) * 128],
                                         rhs=R[:, i * 512:(i + 1) * 512],
                                         start=(i == 0), stop=(i == cpb - 1))
                        nc.tensor.matmul(out=pden[:],
                                         lhsT=OH[:, i * 128:(i + 1) * 128],
                                         rhs=p16[:, i * 8:(i + 1) * 8],
                                         start=(i == 0), stop=(i == cpb - 1))
                    # epilogue: h = elu(num/den), transpose, layer-2 tables
                    dcl = pep.tile([128, 8], f32d, tag="dcl")
                    nc.vector.tensor_scalar_max(dcl[:], pden[:], 1e-30)
                    nc.vector.reciprocal(out=dcl[:], in_=dcl[:])
                    h = pep.tile([128, 512], f32d, tag="h")
                    nc.vector.tensor_tensor(
                        out=h[:].rearrange("p (e f) -> p e f", e=8),
                        in0=pnum[:].rearrange("p (e f) -> p e f", e=8),
                        in1=dcl[:].to_broadcast([128, 8, 64]),
                        op=ALU.mult)
                    hm = pep.tile([128, 512], f32d, tag="hm")
                    nc.vector.tensor_scalar_min(hm[:], h[:], 0.0)
                    nc.scalar.activation(out=hm[:], in_=hm[:], func=AF.Exp)
                    nc.vector.tensor_scalar_sub(hm[:], hm[:], 1.0)
                    nc.vector.tensor_tensor(out=h[:], in0=hm[:], in1=h[:],
                                            op=ALU.max)
                    ps2 = pps.tile([128, 66], f32d, tag="ps2")
                    for k in range(KT):
                        pt = ppt.tile([128, 128], f32d, tag="pt")
                        nc.tensor.transpose(out=pt[:],
                                            in_=h[:, k * 128:(k + 1) * 128],
                                            identity=ident[:])
                        ht = pep.tile([128, 128], f32r, tag="ht")
                        nc.scalar.activation(out=ht[:], in_=pt[:], func=AF.Copy)
                        nc.tensor.matmul(
                            out=ps2[:], lhsT=ht[:],
                            rhs=w2_t[:, k * 66:(k + 1) * 66],
                            start=(k == 0), stop=(k == KT - 1))
                    t2 = pep.tile([128, DW2], f16d, tag="t2")
                    nc.scalar.activation(out=t2[:, 0:64], in_=ps2[:, 0:64],
                                         func=AF.Copy)
                    nc.scalar.activation(out=t2[:, 64:65], in_=ps2[:, 65:66],
                                         func=AF.Copy)
                    nc.vector.tensor_tensor(out=t2[:, 65:66], in0=ps2[:, 65:66],
                                            in1=t2[:, 64:65], op=ALU.subtract)
                    fd2 = pep.tile([128, 2], f16d, tag="fd2")
                    nc.scalar.activation(out=fd2[:, 0:1], in_=ps2[:, 64:65],
                                         func=AF.Copy)
                    nc.vector.tensor_tensor(out=fd2[:, 1:2], in0=ps2[:, 64:65],
                                            in1=fd2[:, 0:1], op=ALU.subtract)
                    nc.sync.dma_start(out=tab2s[rows, :], in_=t2[:])
                    nc.sync.dma_start(out=fdc[rows, :], in_=fd2[:])

            with nc.named_scope("ag2"):
                nc.gpsimd.collective_compute(
                    "AllGather", ALU.bypass, replica_groups=[list(range(NC))],
                    ins=[tab2s[0:LOSH, :].opt()], outs=[tab2a.opt()])
                nc.gpsimd.collective_compute(
                    "AllGather", ALU.bypass, replica_groups=[list(range(NC))],
                    ins=[tab2s[LOSH:SHARD, :].opt()],
                    outs=[tab2b.opt()])

            # ---------------- Phase C ----------------
            with nc.named_scope("phaseC"), \
                 tc.tile_pool(name="pg2", bufs=3) as pg2, \
                 tc.tile_pool(name="pot2", bufs=2) as pot2, \
                 tc.tile_pool(name="pwk2", bufs=2) as pwk2, \
                 tc.tile_pool(name="ppc", bufs=1, space="PSUM") as ppc, \
                 tc.tile_pool(name="ppd2", bufs=2, space="PSUM") as ppd2:
                for bl in range(NBPC):
                    rows = slice(bl * 128, (bl + 1) * 128)
                    ch0 = bl * cpb
                    ps3 = ppc.tile([128, 65], f32d, tag="ps3")
                    G2 = pg2.tile([128, cpb * DW2], f16d, tag="G2")
                    gather_calls(pg2, G2, (tab2a, tab2b), DW2, ch0)
                    OT2 = pot2.tile([128, cpb * 128], f16d, tag="OT2")
                    nc.sync.dma_start(
                        out=OT2[:], in_=ohtd[:, ch0 * 128:(ch0 + cpb) * 128])
                    fd2b = pot2.tile([128, 2], f16d, tag="fd2b")
                    nc.sync.dma_start(out=fd2b[:], in_=fdc[rows, :])
                    D2ps = ppd2.tile([128, cpb], f32d, tag="D2ps")
                    for i in range(cpb):
                        nc.tensor.matmul(out=D2ps[:, i:i + 1],
                                         lhsT=OT2[:, i * 128:(i + 1) * 128],
                                         rhs=fd2b[:, 0:1], start=True, stop=False)
                        nc.tensor.matmul(out=D2ps[:, i:i + 1],
                                         lhsT=OT2[:, i * 128:(i + 1) * 128],
                                         rhs=fd2b[:, 1:2], start=False, stop=True)
                    OH2 = pwk2.tile([128, cpb * 128], f16d, tag="OH2")
                    nc.vector.tensor_tensor(
                        out=OH2[:].rearrange("p (c f) -> p c f", c=cpb),
                        in0=iot[:].rearrange("p (o f) -> p o f", o=1)
                            .to_broadcast([128, cpb, 128]),
                        in1=rl_t[:, ch0:ch0 + cpb].to_broadcast([128, cpb, 128]),
                        op=ALU.is_equal)
                    e2 = pwk2.tile([128, cpb], f32d, tag="e2")
                    t2c = pwk2.tile([128, cpb], f32d, tag="t2c")
                    p2 = pwk2.tile([128, cpb], f16d, tag="p2")
                    G2r = G2[:].rearrange("p (c d) -> p c d", d=DW2)
                    D2r = D2ps[:].rearrange("p (c d) -> p c d", d=1)
                    e2r = e2[:].rearrange("p (c o) -> p c o", o=1)
                    nc.vector.tensor_tensor(out=e2r, in0=G2r[:, :, 64:65],
                                            in1=G2r[:, :, 65:66], op=ALU.add)
                    nc.vector.tensor_tensor(out=e2r, in0=e2r, in1=D2r[:, :, 0:1],
                                            op=ALU.add)
                    nc.vector.tensor_scalar_mul(t2c[:], e2[:], ALPHA)
                    nc.vector.tensor_tensor(out=e2[:], in0=e2[:], in1=t2c[:],
                                            op=ALU.max)
                    nc.scalar.activation(out=p2[:], in_=e2[:], func=AF.Exp)
                    R2 = pwk2.tile([128, cpb * 65], f16d, tag="R2")
                    R2r = R2[:].rearrange("p (c d) -> p c d", d=65)
                    nc.vector.tensor_tensor(
                        out=R2r[:, :, 0:64],
                        in0=G2r[:, :, 0:64],
                        in1=p2[:].rearrange("p (c o) -> p c o", o=1)
                            .to_broadcast([128, cpb, 64]),
                        op=ALU.mult)
                    nc.scalar.activation(
                        out=R2r[:, :, 64:65],
                        in_=p2[:].rearrange("p (c o) -> p c o", o=1),
                        func=AF.Copy)
                    for i in range(cpb):
                        nc.tensor.matmul(out=ps3[:],
                                         lhsT=OH2[:, i * 128:(i + 1) * 128],
                                         rhs=R2[:, i * 65:(i + 1) * 65],
                                         start=(i == 0), stop=(i == cpb - 1))
                    d2c = pwk2.tile([128, 1], f32d, tag="d2c")
                    nc.vector.tensor_scalar_max(d2c[:], ps3[:, 64:65], 1e-30)
                    nc.vector.reciprocal(out=d2c[:], in_=d2c[:])
                    o = pwk2.tile([128, 64], f32d, tag="o")
                    nc.vector.tensor_tensor(
                        out=o[:].rearrange("p (c f) -> p c f", c=1),
                        in0=ps3[:, 0:64].rearrange("p (c f) -> p c f", c=1),
                        in1=d2c[:].to_broadcast([128, 1, 64]),
                        op=ALU.mult)
                    nc.sync.dma_start(out=out[rows, :], in_=o[:])

    nc.compile()
    _BUILD_CACHE[key] = nc
    return nc


def kernel(**inputs):
    global LAST_EXEC_NS, LAST_RESULTS
    x = inputs["x"].astype(np.float32)
    row = inputs["row"].astype(np.int64)
    col = inputs["col"].astype(np.int64)
    W, a = inputs["W"].astype(np.float32), inputs["a"].astype(np.float32)
    W_out = inputs["W_out"].astype(np.float32)
    a_out = inputs["a_out"].astype(np.float32)

    perm, rl, idx16, oht, NLOC, NHIC = _preprocess(row, col)

    W_cat = np.concatenate([W[h] for h in range(NHEADS)], axis=1)
    WA_dst = np.stack([W[h] @ a[h, :NHID] for h in range(NHEADS)], 1)
    WA_src = np.stack([W[h] @ a[h, NHID:] for h in range(NHEADS)], 1)
    w1_np = np.concatenate([W_cat, WA_dst, WA_src], 1).astype(np.float32)
    w2_np = np.concatenate([W_out, (W_out @ a_out[:NCLASS])[:, None],
                            (W_out @ a_out[NCLASS:])[:, None]], 1).astype(np.float32)

    x_pad = np.zeros((NPAD, NFEAT), np.float32)
    x_pad[perm[:N]] = x
    iotar = np.broadcast_to(np.arange(128, dtype=np.float16), (128, 128)).copy()

    nc = _build(NLOC, NHIC)

    in_maps = []
    for c in range(NC):
        xs = x_pad[c * SHARD:(c + 1) * SHARD]            # [6272, 512]
        xtc = (xs.reshape(NBPC, 128, KT, 128)            # [nt, j, k, p]
                 .transpose(0, 3, 2, 1)                  # [nt, p, k, j]
                 .reshape(NBPC, 128, XW)).copy()
        in_maps.append({"xt": xtc, "w1": w1_np, "w2": w2_np,
                        "rl": rl[c], "idx": idx16[c], "ohtd": oht[c],
                        "iotar": iotar})

    trace = bool(int(os.environ.get("GAT_TRACE", "0")))
    res = run_bass_kernel_spmd(nc, in_maps, list(range(NC)), trace=trace,
                               trace_cores=list(range(NC)) if trace else None)
    LAST_EXEC_NS = res.exec_time_ns
    LAST_RESULTS = res
    out_new = np.concatenate([res.results[c]["out"] for c in range(NC)], 0)
    return out_new[perm[:N]].astype(np.float32)


# revision 17
# speedup vs baseline: 1.0867x; 1.0382x over previous
"""GAT (2-layer, 8-head) Trainium2 kernel over 8 NeuronCores.

Strategy (edge-cut node sharding, v5):
- Degree-balanced node relabeling (LPT into 392 blocks of 128). Core c
  owns blocks [49c, 49(c+1)); each block's in-edges are split into
  lo-source (node id < 25088) and hi-source chunks of 128 so gather
  indices fit int16, then padded to global (NLOC, NHIC) chunk counts.
- Edge gathers use the batched SWDGE dma_gather (InstDMAGatherAnt):
  512 indices per instruction, packed descriptors -> ~2x the service
  rate of per-chunk indirect DMAs. Layer-1 table rows are 640 f16
  (1280B, 256B-aligned): [Wh 512 | f_src_hi 8 | f_src_lo 8 | pad].
  Layer-2 rows are 128 f16 (256B): [Wh2 64 | f2_hi | f2_lo | pad].
- Per-edge f_dst comes from a per-chunk PE matmul D = OHT.T @ fd where
  OHT (transposed one-hot) is host-precomputed; OH is built on device
  via is_equal(iota, rl). p=exp(leaky(fsrc+D)); R=p*Wh;
  pnum += OH.T@R, pden += OH.T@p accumulate in PSUM per dst block.
- Phase A computes the layer-1 table with fp32r matmuls (full PE rate);
  AllGather between phases; phase C mirrors B at width 64.
- Softmax needs no segment-max: logits are O(6) so fp16 exp is safe,
  and normalization commutes with the scatter-sum.
"""
import os
import sys
sys.path.insert(0, "/opt/trn_rl_repo")
import numpy as np

import concourse.tile as tile
from concourse import bass, bacc, mybir
from concourse.bass_utils import run_bass_kernel_spmd
from concourse.masks import make_identity

N, E = 50000, 800000
NFEAT, NHID, NHEADS, NCLASS = 512, 64, 8, 64
ALPHA = 0.2
NC = 8
NPAD = 50176
SHARD = NPAD // NC        # 6272
LOSH = 3200               # lo rows per shard (25 blocks)
HISH = SHARD - LOSH       # 3072
LOSZ = NC * LOSH          # 25600 global lo rows
HISZ = NC * HISH          # 24576
BLK = 128
NBPC = SHARD // BLK       # 49 blocks per core
NBLK = NPAD // BLK        # 392 blocks total
KT = NFEAT // 128         # 4 k-tiles
DW1 = 640                 # f16 row: Wh 512 | fsrc_hi 8 | fsrc_lo 8 | pad 112
DW2 = 128                 # f16 row: Wh2 64 | f2_hi | f2_lo | pad 62
XW = KT * 128             # 512
GCH = 4                   # chunks per dma_gather call (512 idxs)

f16d, f32d, f32r, i16d, i32d = (mybir.dt.float16, mybir.dt.float32,
                                mybir.dt.float32r, mybir.dt.int16,
                                mybir.dt.int32)

LAST_EXEC_NS = None
LAST_RESULTS = None
_BUILD_CACHE = {}


def _preprocess(row, col):
    """Degree-balanced relabeling + lo/hi chunk building."""
    import heapq
    deg = np.bincount(row, minlength=NPAD)
    order = np.argsort(-deg, kind="stable")
    heap = [(0, b) for b in range(NBLK)]
    heapq.heapify(heap)
    fill = np.zeros(NBLK, np.int32)
    perm = np.empty(NPAD, np.int64)
    loads = np.zeros(NBLK, np.int64)
    for v in order:
        while True:
            load, b = heapq.heappop(heap)
            if fill[b] < BLK:
                break
        perm[v] = b * BLK + fill[b]
        fill[b] += 1
        loads[b] = load + deg[v]
        if fill[b] < BLK:
            heapq.heappush(heap, (loads[b], b))

    row_n = perm[row]
    col_n = perm[col]
    o = np.argsort(row_n, kind="stable")
    row_s, col_s = row_n[o], col_n[o]
    counts = np.bincount(row_s // BLK, minlength=NBLK)
    starts = np.concatenate([[0], np.cumsum(counts)])

    # per-block lo/hi edge lists
    lo_ids, lo_rls, hi_ids, hi_rls = [], [], [], []
    nlo_max = nhi_max = 0
    for b in range(NBLK):
        s, e = starts[b], starts[b + 1]
        cs = col_s[s:e]
        cc_ = cs // SHARD
        rr_ = cs % SHARD
        tr = np.where(rr_ < LOSH, cc_ * LOSH + rr_,
                      LOSZ + cc_ * HISH + (rr_ - LOSH))
        ls = (row_s[s:e] - b * BLK)
        m = tr < LOSZ
        lo_ids.append(tr[m]); lo_rls.append(ls[m])
        hi_ids.append(tr[~m] - LOSZ); hi_rls.append(ls[~m])
        nlo_max = max(nlo_max, int(m.sum()))
        nhi_max = max(nhi_max, int((~m).sum()))
    NLOC = -(-nlo_max // 128)
    NHIC = -(-nhi_max // 128)
    cpb = NLOC + NHIC
    ncht = NBPC * cpb

    ids_arr = np.zeros((NC, ncht, 128), np.int16)
    rl = np.full((NC, 128, ncht), 200.0, np.float16)
    for b in range(NBLK):
        c, bl = divmod(b, NBPC)
        for ids, rls, chunk0, nch in ((lo_ids[b], lo_rls[b], bl * cpb, NLOC),
                                      (hi_ids[b], hi_rls[b],
                                       bl * cpb + NLOC, NHIC)):
            n = len(ids)
            pad = nch * 128 - n
            idp = np.concatenate([ids, np.zeros(pad, ids.dtype)])
            rlp = np.concatenate([rls, np.full(pad, 200, rls.dtype)])
            ids_arr[c, chunk0:chunk0 + nch] = idp.reshape(nch, 128)
            rl[c, :, chunk0:chunk0 + nch] = rlp.reshape(nch, 128).T
    # wrap: idx16[r, ch*8+g] = ids_arr[ch, g*16+r]
    idx16 = (ids_arr.reshape(NC, ncht, 8, 16)
             .transpose(0, 3, 1, 2)           # [NC, 16, ncht, 8]
             .reshape(NC, 16, ncht * 8))
    idx16 = np.tile(idx16, (1, 8, 1)).astype(np.int16)   # [NC, 128, ncht*8]

    oht = np.zeros((NC, 128, ncht * 128), np.float16)
    rli = rl.astype(np.int32)
    cc, ee, ch = np.nonzero(rli < 128)
    dv = rli[cc, ee, ch]
    oht[cc, dv, ch * 128 + ee] = 1.0
    return perm, rl, idx16, oht, NLOC, NHIC


def _build(NLOC, NHIC):
    key = (NLOC, NHIC)
    if key in _BUILD_CACHE:
        return _BUILD_CACHE[key]
    cpb = NLOC + NHIC
    ncht = NBPC * cpb
    nc = bacc.Bacc("TRN2", target_bir_lowering=False, debug=False,
                   enable_asserts=True, num_devices=NC, num_swdge_queues=4)
    xt = nc.dram_tensor("xt", [NBPC, 128, XW], f32d, kind="ExternalInput")
    w1 = nc.dram_tensor("w1", [KT * 128, 528], f32d, kind="ExternalInput")
    w2 = nc.dram_tensor("w2", [KT * 128, 66], f32d, kind="ExternalInput")
    rl = nc.dram_tensor("rl", [128, ncht], f16d, kind="ExternalInput")
    idx = nc.dram_tensor("idx", [128, ncht * 8], i16d, kind="ExternalInput")
    ohtd = nc.dram_tensor("ohtd", [128, ncht * 128], f16d, kind="ExternalInput")
    iotar_in = nc.dram_tensor("iotar", [128, 128], f16d, kind="ExternalInput")
    out = nc.dram_tensor("out", [SHARD, NCLASS], f32d, kind="ExternalOutput")

    AF, ALU = mybir.ActivationFunctionType, mybir.AluOpType

    def gather_calls(gp, Gt, tabs, dw, ch0):
        """Emit lo+hi dma_gather calls for one block into tile Gt."""
        q = 0
        for tab, nch, goff in ((tabs[0], NLOC, 0), (tabs[1], NHIC, NLOC)):
            for g0 in range(0, nch, GCH):
                k = min(GCH, nch - g0)
                gc = goff + g0                    # chunk within block
                cg = ch0 + gc                     # global chunk
                nc.gpsimd.dma_gather(
                    Gt[:, gc * dw:(gc + k) * dw]
                        .rearrange("p (c d) -> p c d", c=k),
                    tab[:, :],
                    idx_t[:, cg * 8:(cg + k) * 8],
                    k * 128, k * 128, dw, queue_num=q)
                q = (q + 1) % 4

    with tile.TileContext(nc) as tc:
        with tc.tile_pool(name="res", bufs=1) as res, \
             tc.tile_pool(name="dram", bufs=1, space="DRAM") as drp:
            tab1s = drp.tile([SHARD, DW1], f16d)
            tab1a = drp.tile([LOSZ, DW1], f16d, addr_space="Shared")
            tab1b = drp.tile([HISZ, DW1], f16d, addr_space="Shared")
            fda = drp.tile([SHARD, 16], f16d)
            fdc = drp.tile([SHARD, 2], f16d)
            tab2s = drp.tile([SHARD, DW2], f16d)
            tab2a = drp.tile([LOSZ, DW2], f16d, addr_space="Shared")
            tab2b = drp.tile([HISZ, DW2], f16d, addr_space="Shared")

            w1_t = res.tile([128, KT * 528], f32r)
            w2_t = res.tile([128, KT * 66], f32r)
            for k in range(KT):
                nc.sync.dma_start(out=w1_t[:, k * 528:(k + 1) * 528],
                                  in_=w1[k * 128:(k + 1) * 128, :].bitcast(f32r))
                nc.sync.dma_start(out=w2_t[:, k * 66:(k + 1) * 66],
                                  in_=w2[k * 128:(k + 1) * 128, :].bitcast(f32r))
            rl_t = res.tile([128, ncht], f16d)
            nc.sync.dma_start(out=rl_t[:], in_=rl[:, :])
            idx_t = res.tile([128, ncht * 8], i16d)
            nc.sync.dma_start(out=idx_t[:], in_=idx[:, :])
            iot = res.tile([128, 128], f16d)
            nc.sync.dma_start(out=iot[:], in_=iotar_in[:, :])
            ident = res.tile([128, 128], f32d)
            make_identity(nc, ident[:])

            # ---------------- Phase A ----------------
            with nc.named_scope("phaseA"), \
                 tc.tile_pool(name="pa", bufs=3) as pa, \
                 tc.tile_pool(name="ppa", bufs=2, space="PSUM") as ppa:
                for nt in range(NBPC):
                    rows = slice(nt * 128, (nt + 1) * 128)
                    psA = ppa.tile([128, 512], f32d, tag="psA")
                    psB = ppa.tile([128, 16], f32d, tag="psB")
                    XK = pa.tile([128, XW], f32r, tag="XK")
                    nc.sync.dma_start(out=XK[:], in_=xt[nt, :, :].bitcast(f32r))
                    for k in range(KT):
                        nc.tensor.matmul(
                            out=psA[:], lhsT=XK[:, k * 128:(k + 1) * 128],
                            rhs=w1_t[:, k * 528:k * 528 + 512],
                            start=(k == 0), stop=(k == KT - 1))
                        nc.tensor.matmul(
                            out=psB[:], lhsT=XK[:, k * 128:(k + 1) * 128],
                            rhs=w1_t[:, k * 528 + 512:(k + 1) * 528],
                            start=(k == 0), stop=(k == KT - 1))
                    whf = pa.tile([128, DW1], f16d, tag="whf")
                    nc.scalar.activation(out=whf[:, :512], in_=psA[:],
                                         func=AF.Copy)
                    nc.scalar.activation(out=whf[:, 512:520], in_=psB[:, 8:16],
                                         func=AF.Copy)
                    nc.vector.tensor_tensor(out=whf[:, 520:528], in0=psB[:, 8:16],
                                            in1=whf[:, 512:520], op=ALU.subtract)
                    fdt = pa.tile([128, 16], f16d, tag="fdt")
                    nc.scalar.activation(out=fdt[:, 0:8], in_=psB[:, 0:8],
                                         func=AF.Copy)
                    nc.vector.tensor_tensor(out=fdt[:, 8:16], in0=psB[:, 0:8],
                                            in1=fdt[:, 0:8], op=ALU.subtract)
                    nc.sync.dma_start(out=tab1s[rows, :], in_=whf[:])
                    nc.sync.dma_start(out=fda[rows, :], in_=fdt[:])

            with nc.named_scope("ag1"):
                nc.gpsimd.collective_compute(
                    "AllGather", ALU.bypass, replica_groups=[list(range(NC))],
                    ins=[tab1s[0:LOSH, :].opt()], outs=[tab1a.opt()])
                nc.gpsimd.collective_compute(
                    "AllGather", ALU.bypass, replica_groups=[list(range(NC))],
                    ins=[tab1s[LOSH:SHARD, :].opt()],
                    outs=[tab1b.opt()])

            # ---------------- Phase B ----------------
            with nc.named_scope("phaseB"), \
                 tc.tile_pool(name="pg", bufs=3) as pg, \
                 tc.tile_pool(name="pot", bufs=2) as pot, \
                 tc.tile_pool(name="pwk", bufs=2) as pwk, \
                 tc.tile_pool(name="pep", bufs=2) as pep, \
                 tc.tile_pool(name="ppb", bufs=1, space="PSUM") as ppb, \
                 tc.tile_pool(name="ppd", bufs=2, space="PSUM") as ppd, \
                 tc.tile_pool(name="ppt", bufs=2, space="PSUM") as ppt, \
                 tc.tile_pool(name="pps", bufs=1, space="PSUM") as pps:
                for bl in range(NBPC):
                    rows = slice(bl * 128, (bl + 1) * 128)
                    ch0 = bl * cpb
                    pnum = ppb.tile([128, 512], f32d, tag="pnum")
                    pden = ppb.tile([128, 8], f32d, tag="pden")
                    G = pg.tile([128, cpb * DW1], f16d, tag="G")
                    gather_calls(pg, G, (tab1a, tab1b), DW1, ch0)
                    OT = pot.tile([128, cpb * 128], f16d, tag="OT")
                    nc.sync.dma_start(
                        out=OT[:], in_=ohtd[:, ch0 * 128:(ch0 + cpb) * 128])
                    fdb = pot.tile([128, 16], f16d, tag="fdb")
                    nc.sync.dma_start(out=fdb[:], in_=fda[rows, :])
                    Dps = ppd.tile([128, cpb * 8], f32d, tag="Dps")
                    for i in range(cpb):
                        nc.tensor.matmul(out=Dps[:, i * 8:(i + 1) * 8],
                                         lhsT=OT[:, i * 128:(i + 1) * 128],
                                         rhs=fdb[:, 0:8], start=True, stop=False)
                        nc.tensor.matmul(out=Dps[:, i * 8:(i + 1) * 8],
                                         lhsT=OT[:, i * 128:(i + 1) * 128],
                                         rhs=fdb[:, 8:16], start=False, stop=True)
                    OH = pwk.tile([128, cpb * 128], f16d, tag="OH")
                    nc.vector.tensor_tensor(
                        out=OH[:].rearrange("p (c f) -> p c f", c=cpb),
                        in0=iot[:].rearrange("p (o f) -> p o f", o=1)
                            .to_broadcast([128, cpb, 128]),
                        in1=rl_t[:, ch0:ch0 + cpb].to_broadcast([128, cpb, 128]),
                        op=ALU.is_equal)
                    s1 = pwk.tile([128, cpb * 8], f32d, tag="s1")
                    e1 = pwk.tile([128, cpb * 8], f32d, tag="e1")
                    p16 = pwk.tile([128, cpb * 8], f16d, tag="p16")
                    Gq = G[:].rearrange("p (c d) -> p c d", d=DW1)
                    Dr = Dps[:].rearrange("p (c d) -> p c d", d=8)
                    s1r = s1[:].rearrange("p (c f) -> p c f", c=cpb)
                    e1r = e1[:].rearrange("p (c f) -> p c f", c=cpb)
                    nc.vector.tensor_tensor(out=s1r, in0=Gq[:, :, 512:520],
                                            in1=Gq[:, :, 520:528], op=ALU.add)
                    nc.vector.tensor_tensor(out=e1r, in0=s1r, in1=Dr[:, :, 0:8],
                                            op=ALU.add)
                    nc.vector.tensor_scalar_mul(s1[:], e1[:], ALPHA)
                    nc.vector.tensor_tensor(out=e1[:], in0=e1[:], in1=s1[:],
                                            op=ALU.max)
                    nc.scalar.activation(out=p16[:], in_=e1[:], func=AF.Exp)
                    R = pwk.tile([128, cpb * 512], f16d, tag="R")
                    nc.vector.tensor_tensor(
                        out=R[:].rearrange("p (c e f) -> p c e f", c=cpb, e=8),
                        in0=Gq[:, :, 0:512].rearrange("p c (e f) -> p c e f", e=8),
                        in1=p16[:].rearrange("p (c h) -> p c h", c=cpb)
                            .to_broadcast([128, cpb, 8, 64]),
                        op=ALU.mult)
                    for i in range(cpb):
                        nc.tensor.matmul(out=pnum[:],
                                         lhsT=OH[:, i * 128:(i + 1) * 128],
                                         rhs=R[:, i * 512:(i + 1) * 512],
                                         start=(i == 0), stop=(i == cpb - 1))
                        nc.tensor.matmul(out=pden[:],
                                         lhsT=OH[:, i * 128:(i + 1) * 128],
                                         rhs=p16[:, i * 8:(i + 1) * 8],
                                         start=(i == 0), stop=(i == cpb - 1))
                    # epilogue: h = elu(num/den), transpose, layer-2 tables
                    dcl = pep.tile([128, 8], f32d, tag="dcl")
                    nc.vector.tensor_scalar_max(dcl[:], pden[:], 1e-30)
                    nc.vector.reciprocal(out=dcl[:], in_=dcl[:])
                    h = pep.tile([128, 512], f32d, tag="h")
                    nc.vector.tensor_tensor(
                        out=h[:].rearrange("p (e f) -> p e f", e=8),
                        in0=pnum[:].rearrange("p (e f) -> p e f", e=8),
                        in1=dcl[:].to_broadcast([128, 8, 64]),
                        op=ALU.mult)
                    hm = pep.tile([128, 512], f32d, tag="hm")
                    nc.vector.tensor_scalar_min(hm[:], h[:], 0.0)
                    nc.scalar.activation(out=hm[:], in_=hm[:], func=AF.Exp)
                    nc.vector.tensor_scalar_sub(hm[:], hm[:], 1.0)
                    nc.vector.tensor_tensor(out=h[:], in0=hm[:], in1=h[:],
                                            op=ALU.max)
                    ps2 = pps.tile([128, 66], f32d, tag="ps2")
                    for k in range(KT):
                        pt = ppt.tile([128, 128], f32d, tag="pt")
                        nc.tensor.transpose(out=pt[:],
                                            in_=h[:, k * 128:(k + 1) * 128],
                                            identity=ident[:])
                        ht = pep.tile([128, 128], f32r, tag="ht")
                        nc.scalar.activation(out=ht[:], in_=pt[:], func=AF.Copy)
                        nc.tensor.matmul(
                            out=ps2[:], lhsT=ht[:],
                            rhs=w2_t[:, k * 66:(k + 1) * 66],
                            start=(k == 0), stop=(k == KT - 1))
                    t2 = pep.tile([128, DW2], f16d, tag="t2")
                    nc.scalar.activation(out=t2[:, 0:64], in_=ps2[:, 0:64],
                                         func=AF.Copy)
                    nc.scalar.activation(out=t2[:, 64:65], in_=ps2[:, 65:66],
                                         func=AF.Copy)
                    nc.vector.tensor_tensor(out=t2[:, 65:66], in0=ps2[:, 65:66],
                                            in1=t2[:, 64:65], op=ALU.subtract)
                    fd2 = pep.tile([128, 2], f16d, tag="fd2")
                    nc.scalar.activation(out=fd2[:, 0:1], in_=ps2[:, 64:65],
                                         func=AF.Copy)
                    nc.vector.tensor_tensor(out=fd2[:, 1:2], in0=ps2[:, 64:65],
                                            in1=fd2[:, 0:1], op=ALU.subtract)
                    nc.sync.dma_start(out=tab2s[rows, :], in_=t2[:])
                    nc.sync.dma_start(out=fdc[rows, :], in_=fd2[:])

            with nc.named_scope("ag2"):
                nc.gpsimd.collective_compute(
                    "AllGather", ALU.bypass, replica_groups=[list(range(NC))],
                    ins=[tab2s[0:LOSH, :].opt()], outs=[tab2a.opt()])
                nc.gpsimd.collective_compute(
                    "AllGather", ALU.bypass, replica_groups=[list(range(NC))],
                    ins=[tab2s[LOSH:SHARD, :].opt()],
                    outs=[tab2b.opt()])

            # ---------------- Phase C ----------------
            with nc.named_scope("phaseC"), \
                 tc.tile_pool(name="pg2", bufs=3) as pg2, \
                 tc.tile_pool(name="pot2", bufs=2) as pot2, \
                 tc.tile_pool(name="pwk2", bufs=2) as pwk2, \
                 tc.tile_pool(name="ppc", bufs=1, space="PSUM") as ppc, \
                 tc.tile_pool(name="ppd2", bufs=2, space="PSUM") as ppd2:
                for bl in range(NBPC):
                    rows = slice(bl * 128, (bl + 1) * 128)
                    ch0 = bl * cpb
                    ps3 = ppc.tile([128, 65], f32d, tag="ps3")
                    G2 = pg2.tile([128, cpb * DW2], f16d, tag="G2")
                    gather_calls(pg2, G2, (tab2a, tab2b), DW2, ch0)
                    OT2 = pot2.tile([128, cpb * 128], f16d, tag="OT2")
                    nc.sync.dma_start(
                        out=OT2[:], in_=ohtd[:, ch0 * 128:(ch0 + cpb) * 128])
                    fd2b = pot2.tile([128, 2], f16d, tag="fd2b")
                    nc.sync.dma_start(out=fd2b[:], in_=fdc[rows, :])
                    D2ps = ppd2.tile([128, cpb * 2], f32d, tag="D2ps")
                    for i in range(cpb):
                        nc.tensor.matmul(out=D2ps[:, i * 2:(i + 1) * 2],
                                         lhsT=OT2[:, i * 128:(i + 1) * 128],
                                         rhs=fd2b[:], start=True, stop=True)
                    OH2 = pwk2.tile([128, cpb * 128], f16d, tag="OH2")
                    nc.vector.tensor_tensor(
                        out=OH2[:].rearrange("p (c f) -> p c f", c=cpb),
                        in0=iot[:].rearrange("p (o f) -> p o f", o=1)
                            .to_broadcast([128, cpb, 128]),
                        in1=rl_t[:, ch0:ch0 + cpb].to_broadcast([128, cpb, 128]),
                        op=ALU.is_equal)
                    e2 = pwk2.tile([128, cpb], f32d, tag="e2")
                    t2c = pwk2.tile([128, cpb], f32d, tag="t2c")
                    p2 = pwk2.tile([128, cpb], f16d, tag="p2")
                    G2r = G2[:].rearrange("p (c d) -> p c d", d=DW2)
                    D2r = D2ps[:].rearrange("p (c d) -> p c d", d=2)
                    e2r = e2[:].rearrange("p (c o) -> p c o", o=1)
                    nc.vector.tensor_tensor(out=e2r, in0=G2r[:, :, 64:65],
                                            in1=G2r[:, :, 65:66], op=ALU.add)
                    nc.vector.tensor_tensor(out=e2r, in0=e2r, in1=D2r[:, :, 0:1],
                                            op=ALU.add)
                    nc.vector.tensor_tensor(out=e2r, in0=e2r, in1=D2r[:, :, 1:2],
                                            op=ALU.add)
                    nc.vector.tensor_scalar_mul(t2c[:], e2[:], ALPHA)
                    nc.vector.tensor_tensor(out=e2[:], in0=e2[:], in1=t2c[:],
                                            op=ALU.max)
                    nc.scalar.activation(out=p2[:], in_=e2[:], func=AF.Exp)
                    R2 = pwk2.tile([128, cpb * 65], f16d, tag="R2")
                    R2r = R2[:].rearrange("p (c d) -> p c d", d=65)
                    nc.vector.tensor_tensor(
                        out=R2r[:, :, 0:64],
                        in0=G2r[:, :, 0:64],
                        in1=p2[:].rearrange("p (c o) -> p c o", o=1)
                            .to_broadcast([128, cpb, 64]),
                        op=ALU.mult)
                    nc.scalar.activation(
                        out=R2r[:, :, 64:65],
                        in_=p2[:].rearrange("p (c o) -> p c o", o=1),
                        func=AF.Copy)
                    for i in range(cpb):
                        nc.tensor.matmul(out=ps3[:],
                                         lhsT=OH2[:, i * 128:(i + 1) * 128],
                                         rhs=R2[:, i * 65:(i + 1) * 65],
                                         start=(i == 0), stop=(i == cpb - 1))
                    d2c = pwk2.tile([128, 1], f32d, tag="d2c")
                    nc.vector.tensor_scalar_max(d2c[:], ps3[:, 64:65], 1e-30)
                    nc.vector.reciprocal(out=d2c[:], in_=d2c[:])
                    o = pwk2.tile([128, 64], f32d, tag="o")
                    nc.vector.tensor_tensor(
                        out=o[:].rearrange("p (c f) -> p c f", c=1),
                        in0=ps3[:, 0:64].rearrange("p (c f) -> p c f", c=1),
                        in1=d2c[:].to_broadcast([128, 1, 64]),
                        op=ALU.mult)
                    nc.sync.dma_start(out=out[rows, :], in_=o[:])

    nc.compile()
    _BUILD_CACHE[key] = nc
    return nc


def kernel(**inputs):
    global LAST_EXEC_NS, LAST_RESULTS
    x = inputs["x"].astype(np.float32)
    row = inputs["row"].astype(np.int64)
    col = inputs["col"].astype(np.int64)
    W, a = inputs["W"].astype(np.float32), inputs["a"].astype(np.float32)
    W_out = inputs["W_out"].astype(np.float32)
    a_out = inputs["a_out"].astype(np.float32)

    perm, rl, idx16, oht, NLOC, NHIC = _preprocess(row, col)

    W_cat = np.concatenate([W[h] for h in range(NHEADS)], axis=1)
    WA_dst = np.stack([W[h] @ a[h, :NHID] for h in range(NHEADS)], 1)
    WA_src = np.stack([W[h] @ a[h, NHID:] for h in range(NHEADS)], 1)
    w1_np = np.concatenate([W_cat, WA_dst, WA_src], 1).astype(np.float32)
    w2_np = np.concatenate([W_out, (W_out @ a_out[:NCLASS])[:, None],
                            (W_out @ a_out[NCLASS:])[:, None]], 1).astype(np.float32)

    x_pad = np.zeros((NPAD, NFEAT), np.float32)
    x_pad[perm[:N]] = x
    iotar = np.broadcast_to(np.arange(128, dtype=np.float16), (128, 128)).copy()

    nc = _build(NLOC, NHIC)

    in_maps = []
    for c in range(NC):
        xs = x_pad[c * SHARD:(c + 1) * SHARD]            # [6272, 512]
        xtc = (xs.reshape(NBPC, 128, KT, 128)            # [nt, j, k, p]
                 .transpose(0, 3, 2, 1)                  # [nt, p, k, j]
                 .reshape(NBPC, 128, XW)).copy()
        in_maps.append({"xt": xtc, "w1": w1_np, "w2": w2_np,
                        "rl": rl[c], "idx": idx16[c], "ohtd": oht[c],
                        "iotar": iotar})

    trace = bool(int(os.environ.get("GAT_TRACE", "0")))
    res = run_bass_kernel_spmd(nc, in_maps, list(range(NC)), trace=trace,
                               trace_cores=list(range(NC)) if trace else None)
    LAST_EXEC_NS = res.exec_time_ns
    LAST_RESULTS = res
    out_new = np.concatenate([res.results[c]["out"] for c in range(NC)], 0)
    return out_new[perm[:N]].astype(np.float32)
